# revision 8
# baseline (speedup 1.0000x reference)
"""ANI-AEV-with-bond-order kernel for 8 Trainium2 NeuronCores (Bass/Tile).

Strategy
--------
Host (sharding/unsharding, pure index math + layout):
  * Each core owns a contiguous range of 6250 atoms.
  * Radial edges are routed to the core owning edge_src; the scatter target
    table has one width-16 row per (atom, species_dst, bond_bit).
  * Angular pairs are routed to the core owning central_atom; table rows are
    (atom, pair_species), width 16.
  * Rows are laid out in a padded "(group, window, partition, j)" structure:
    a group is 32 windows x 128 partitions = 4096 rows sharing a slot count K
    (rows sorted by count; heavy rows split into virtual rows of <= CAP slots,
    partials merged on unshard).  Device slot (g,p,j,w) holds the j-th item of
    row (g,w,p).

Device (all FLOPs + the segment reduction):
  * Radial, per group:  qt_r = f16(d - s_r); q = qt*qt;
    e = bf16(exp(-16 q)) [ACT]; u = e * (0.25*switch) [bf16];
    table rows accumulate u over j via PE identity-matmuls into PSUM
    (out += I.T @ u_j), one 512-wide matmul per j — the segment sum costs
    ~0.25 PE-cycles/slot.
  * Angular, per group:  with q_z=(theta-sZ)^2, the reference factor
    (0.5+0.5cos(theta-sZ))^32 = cos^64((theta-sZ)/2) is evaluated as
    exp(-(8q + q^2/3)) (error < 0.3%); combined with the distance gaussian and
    the switch product inside one exp:
      X = [qz*(qz/3+8)]_z + [8(d12-sA)^2 - ln(ss*st)]_a
      f = exp(-X + ln2) = 2*ss*st*cos^64(..)*exp(-8(d12-sA)^2)
    then the same PE identity-matmul accumulation.
"""

import os
import numpy as np
import ml_dtypes

import concourse.bass as bass
import concourse.bacc as bacc
import concourse.mybir as mybir
import concourse.tile as tile
from concourse.masks import make_identity
from concourse.bass_utils import run_bass_kernel_spmd

BF16 = ml_dtypes.bfloat16
F16D = mybir.dt.float16
BF16D = mybir.dt.bfloat16
F32D = mybir.dt.float32

# ---- problem constants (hardcoded; must match the reference) ----
N_ATOMS = 50000
E_RAD = 2000000
E_ANG = 1000000
N_PAIRS = 2000000
NUM_SPECIES = 4
ECFP_DIM = 16
RADIAL_ETA = 16.0
ANGULAR_ETA = 8.0
RADIAL_DIV = 16
ANGULAR_DIV = 4
ZETA = 32.0
ANGLE_SECTIONS = 4
RADIAL_START = 0.8
ANGULAR_START = 0.8
CUTOFF = 5.2
ANG_CUTOFF = 3.5
NUM_PAIR = NUM_SPECIES * (NUM_SPECIES + 1) // 2

N_CORES = 8
ATOMS_PER_CORE = N_ATOMS // N_CORES
RAD_ROWS = ATOMS_PER_CORE * NUM_SPECIES * 2            # width-16 rows per core
ANG_ROWS = ATOMS_PER_CORE * NUM_PAIR
WINDOWS_PER_GROUP = 32
ROWS_PER_GROUP = 128 * WINDOWS_PER_GROUP               # 4096
CAP = 16

SQRT8 = float(np.sqrt(8.0))
LN2 = float(np.log(2.0))

RAD_SHIFTS = np.linspace(RADIAL_START, CUTOFF, RADIAL_DIV + 1)[:-1].astype(np.float32)
ANG_SHIFTS_Z = (np.linspace(0, np.pi, ANGLE_SECTIONS + 1)
                + np.pi / (2 * ANGLE_SECTIONS))[:-1].astype(np.float32)
ANG_SHIFTS_A = np.linspace(ANGULAR_START, ANG_CUTOFF, ANGULAR_DIV + 1)[:-1].astype(np.float32)


# --------------------------------------------------------------------------
# host-side layout planning
# --------------------------------------------------------------------------

def _plan(rows, n_rows, cap=CAP):
    """Split heavy rows into virtual rows (<= cap items), sort by count."""
    counts = np.bincount(rows, minlength=n_rows)
    n_virt = -(-counts // cap)
    vrow_base = np.concatenate([[0], np.cumsum(n_virt)]).astype(np.int64)
    n_vrows = int(vrow_base[-1])
    item_order = np.argsort(rows, kind="stable")
    sorted_rows = rows[item_order]
    seq = np.arange(len(rows), dtype=np.int64) - np.repeat(
        np.concatenate([[0], np.cumsum(counts)])[:-1], counts)
    vrow_of_item = np.empty(len(rows), dtype=np.int64)
    j_of_item = np.empty(len(rows), dtype=np.int64)
    vrow_of_item[item_order] = vrow_base[sorted_rows] + seq // cap
    j_of_item[item_order] = seq % cap
    vcounts = np.bincount(vrow_of_item, minlength=n_vrows)
    vrow_real = np.repeat(np.arange(n_rows, dtype=np.int64), n_virt)
    order = np.argsort(-vcounts, kind="stable")
    n_groups = (n_vrows + ROWS_PER_GROUP - 1) // ROWS_PER_GROUP
    ks = [int(vcounts[order[g * ROWS_PER_GROUP:(g + 1) * ROWS_PER_GROUP]].max())
          for g in range(n_groups)]
    return dict(vrow_of_item=vrow_of_item, j_of_item=j_of_item,
                vrow_real=vrow_real, order=order, ks=ks, n_vrows=n_vrows)


def _slots(plan, ks):
    """Flat slot index per item for shared group Ks.  Layout: concat over
    groups of (128, K_g, 32) blocks; slot(g,p,j,w)."""
    order = plan["order"]
    n_vrows = plan["n_vrows"]
    n_groups = len(ks)
    bases = np.zeros(n_groups + 1, dtype=np.int64)
    for g in range(n_groups):
        bases[g + 1] = bases[g] + 128 * ks[g] * 32
    vrow_g = np.empty(n_vrows, dtype=np.int64)
    vrow_p = np.empty(n_vrows, dtype=np.int64)
    vrow_w = np.empty(n_vrows, dtype=np.int64)
    idx = np.arange(len(order))
    vrow_g[order] = idx // ROWS_PER_GROUP
    within = idx % ROWS_PER_GROUP
    vrow_w[order] = within // 128
    vrow_p[order] = within % 128
    v = plan["vrow_of_item"]
    K = np.asarray(ks, dtype=np.int64)
    g = vrow_g[v]
    slot = bases[g] + vrow_p[v] * (K[g] * 32) + plan["j_of_item"] * 32 + vrow_w[v]
    return slot, int(bases[-1])


def _unshard_table(dev_out, plan, ks, n_rows):
    """Device output (n_groups*128*512 bf16, layout (g,p,w,r)) -> (n_rows,16) f32."""
    n_groups = len(ks)
    blk = dev_out.astype(np.float32).reshape(n_groups, 128, 32, 16)
    posview = blk.transpose(0, 2, 1, 3).reshape(n_groups * ROWS_PER_GROUP, 16)
    order = plan["order"]
    table = np.zeros((n_rows, 16), dtype=np.float32)
    np.add.at(table, plan["vrow_real"][order], posview[:len(order)])
    return table


# --------------------------------------------------------------------------
# bass kernel builder
# --------------------------------------------------------------------------

def build_kernel(rad_ks, ang_ks):
    nc = bacc.Bacc(None)
    rad_total = 128 * 32 * int(np.sum(rad_ks))
    ang_total = 128 * 32 * int(np.sum(ang_ks))
    rad_d = nc.declare_dram_parameter("rad_d", [rad_total], F32D, isOutput=False)
    rad_c = nc.declare_dram_parameter("rad_c", [rad_total], BF16D, isOutput=False)
    ang_th = nc.declare_dram_parameter("ang_th", [ang_total], F32D, isOutput=False)
    ang_ds = nc.declare_dram_parameter("ang_ds", [ang_total], F32D, isOutput=False)
    ang_dt = nc.declare_dram_parameter("ang_dt", [ang_total], F32D, isOutput=False)
    ang_w = nc.declare_dram_parameter("ang_w", [ang_total], BF16D, isOutput=False)
    rad_out = nc.declare_dram_parameter(
        "rad_out", [len(rad_ks) * 128 * 512], BF16D, isOutput=True)
    ang_out = nc.declare_dram_parameter(
        "ang_out", [len(ang_ks) * 128 * 512], BF16D, isOutput=True)

    with tile.TileContext(nc) as tc:
        with tc.tile_pool(name="const", bufs=1) as cpool:
            ident = cpool.tile([128, 128], BF16D)
            make_identity(nc, ident[:])
            ln2_t = cpool.tile([128, 1], F32D)
            nc.gpsimd.memset(ln2_t[:], LN2)

            # ---------------- radial ----------------
            with tc.tile_pool(name="rin", bufs=3) as rin, \
                 tc.tile_pool(name="rwork", bufs=2) as rwork, \
                 tc.tile_pool(name="rout", bufs=3) as routp, \
                 tc.tile_pool(name="rpsum", bufs=4, space="PSUM") as rpsum:
                base = 0
                for g, K in enumerate(rad_ks):
                    n = 128 * K * 32
                    d_t = rin.tile([128, K * 32], F32D, tag="d")
                    c_t = rin.tile([128, K * 32], BF16D, tag="c")
                    nc.sync.dma_start(
                        out=d_t[:], in_=rad_d[base:base + n].rearrange("(p f) -> p f", p=128))
                    nc.sync.dma_start(
                        out=c_t[:], in_=rad_c[base:base + n].rearrange("(p f) -> p f", p=128))
                    base += n

                    # qt[p, j, r, w] = f16(d[p,j,w] - s_r)
                    qt = rwork.tile([128, K * 16 * 32], F16D, tag="qt")
                    qt4 = qt[:].rearrange("p (j r w) -> p j r w", j=K, r=16)
                    d3 = d_t[:].rearrange("p (j one w) -> p j one w", one=1, w=32)
                    for r in range(RADIAL_DIV):
                        nc.vector.tensor_scalar(
                            out=qt4[:, :, r, :], in0=d3[:, :, 0, :],
                            scalar1=-float(RAD_SHIFTS[r]), scalar2=None,
                            op0=mybir.AluOpType.add)
                    # q = qt*qt  (in place)
                    nc.vector.tensor_tensor(
                        out=qt[:], in0=qt[:], in1=qt[:], op=mybir.AluOpType.mult)
                    # e = bf16(exp(-eta * q))
                    e_t = rwork.tile([128, K * 16 * 32], BF16D, tag="e")
                    nc.scalar.activation(
                        out=e_t[:], in_=qt[:], func=mybir.ActivationFunctionType.Exp,
                        scale=-RADIAL_ETA)
                    # u = e * c   (broadcast c over r; in place on e)
                    c4 = c_t[:].rearrange("p (j one w) -> p j one w", one=1, w=32) \
                        .to_broadcast([128, K, 16, 32])
                    e4 = e_t[:].rearrange("p (j r w) -> p j r w", j=K, r=16)
                    nc.vector.tensor_tensor(
                        out=e4[:], in0=e4[:], in1=c4[:], op=mybir.AluOpType.mult)
                    # PSUM accumulate over j
                    acc = rpsum.tile([128, 512], F32D, tag="acc")
                    e3 = e_t[:].rearrange("p (j f) -> p j f", j=K)
                    for j in range(K):
                        nc.tensor.matmul(
                            out=acc[:], lhsT=ident[:], rhs=e3[:, j, :],
                            start=(j == 0), stop=(j == K - 1))
                    # flush: psum (p, r*32+w) -> sbuf (p, w*16+r) bf16
                    o_t = routp.tile([128, 512], BF16D, tag="o")
                    accv = acc[:].rearrange("p (r w) -> p w r", r=16)
                    ov = o_t[:].rearrange("p (w r) -> p w r", w=32)
                    nc.scalar.activation(
                        out=ov[:], in_=accv[:], func=mybir.ActivationFunctionType.Copy)
                    nc.sync.dma_start(
                        out=rad_out[g * 65536:(g + 1) * 65536].rearrange("(p f) -> p f", p=128),
                        in_=o_t[:])

            # ---------------- angular ----------------
            with tc.tile_pool(name="ain", bufs=3) as ain, \
                 tc.tile_pool(name="awork", bufs=2) as awork, \
                 tc.tile_pool(name="aout", bufs=3) as aoutp, \
                 tc.tile_pool(name="apsum", bufs=4, space="PSUM") as apsum:
                base = 0
                for g, K in enumerate(ang_ks):
                    n = 128 * K * 32
                    th_t = ain.tile([128, K * 32], F32D, tag="th")
                    ds_t = ain.tile([128, K * 32], F32D, tag="ds")
                    dt_t = ain.tile([128, K * 32], F32D, tag="dt")
                    w_t = ain.tile([128, K * 32], BF16D, tag="w")
                    for t, src in ((th_t, ang_th), (ds_t, ang_ds),
                                   (dt_t, ang_dt), (w_t, ang_w)):
                        nc.sync.dma_start(
                            out=t[:], in_=src[base:base + n].rearrange("(p f) -> p f", p=128))
                    base += n

                    # lnw = f16(ln(w))
                    lnw = awork.tile([128, K * 32], F16D, tag="lnw")
                    nc.scalar.activation(
                        out=lnw[:], in_=w_t[:], func=mybir.ActivationFunctionType.Ln)
                    # u12 = f16(ds + dt)
                    u12 = awork.tile([128, K * 32], F16D, tag="u12")
                    nc.vector.tensor_tensor(
                        out=u12[:], in0=ds_t[:], in1=dt_t[:], op=mybir.AluOpType.add)
                    # ta[p,a,j,w] = u12*sqrt8/2 - sqrt8*sA_a ; qa = ta*ta
                    ta = awork.tile([128, 4 * K * 32], F16D, tag="ta")
                    ta4 = ta[:].rearrange("p (a j w) -> p a j w", a=4, j=K)
                    u3 = u12[:].rearrange("p (j w) -> p j w", j=K)
                    for a in range(ANGULAR_DIV):
                        nc.vector.tensor_scalar(
                            out=ta4[:, a, :, :], in0=u3[:],
                            scalar1=SQRT8 / 2.0, scalar2=-SQRT8 * float(ANG_SHIFTS_A[a]),
                            op0=mybir.AluOpType.mult, op1=mybir.AluOpType.add)
                    nc.vector.tensor_tensor(
                        out=ta[:], in0=ta[:], in1=ta[:], op=mybir.AluOpType.mult)
                    # P2[p,a,j,w] = qa - lnw
                    lnwb = lnw[:].rearrange("p (one j w) -> p one j w", one=1, j=K) \
                        .to_broadcast([128, 4, K, 32])
                    nc.vector.tensor_tensor(
                        out=ta4[:], in0=ta4[:], in1=lnwb[:], op=mybir.AluOpType.subtract)
                    # tz[p,z,j,w] = th - sZ_z ; qz = tz*tz
                    tz = awork.tile([128, 4 * K * 32], F16D, tag="tz")
                    tz4 = tz[:].rearrange("p (z j w) -> p z j w", z=4, j=K)
                    th3 = th_t[:].rearrange("p (j w) -> p j w", j=K)
                    for z in range(ANGLE_SECTIONS):
                        nc.vector.tensor_scalar(
                            out=tz4[:, z, :, :], in0=th3[:],
                            scalar1=-float(ANG_SHIFTS_Z[z]), scalar2=None,
                            op0=mybir.AluOpType.add)
                    nc.vector.tensor_tensor(
                        out=tz[:], in0=tz[:], in1=tz[:], op=mybir.AluOpType.mult)
                    # P1 = qz*(qz/3 + 8):  h = qz/3+8 ; P1 = h*qz  (P1 into h)
                    h = awork.tile([128, 4 * K * 32], F16D, tag="h")
                    nc.vector.tensor_scalar(
                        out=h[:], in0=tz[:], scalar1=1.0 / 3.0, scalar2=8.0,
                        op0=mybir.AluOpType.mult, op1=mybir.AluOpType.add)
                    nc.vector.tensor_tensor(
                        out=h[:], in0=h[:], in1=tz[:], op=mybir.AluOpType.mult)
                    # X[p,j,a,z,w] = P1[p,z,j,w] + P2[p,a,j,w]
                    X = awork.tile([128, K * 512], F16D, tag="X")
                    X5 = X[:].rearrange("p (j a z w) -> p j a z w", j=K, a=4, z=4)
                    p1v = h[:].rearrange("p (z j w) -> p j z w", z=4, j=K)
                    p2v = ta[:].rearrange("p (a j one w) -> p a j one w",
                                          a=4, one=1, j=K) \
                        .to_broadcast([128, 4, K, 4, 32])
                    for a in range(ANGULAR_DIV):
                        nc.vector.tensor_tensor(
                            out=X5[:, :, a, :, :], in0=p1v[:],
                            in1=p2v[:, a], op=mybir.AluOpType.add)
                    # f = bf16(exp(-X + ln2))
                    f_t = awork.tile([128, K * 512], BF16D, tag="f")
                    nc.scalar.activation(
                        out=f_t[:], in_=X[:], func=mybir.ActivationFunctionType.Exp,
                        scale=-1.0, bias=ln2_t[:])
                    # PSUM accumulate over j
                    acc = apsum.tile([128, 512], F32D, tag="acc")
                    f3 = f_t[:].rearrange("p (j f) -> p j f", j=K)
                    for j in range(K):
                        nc.tensor.matmul(
                            out=acc[:], lhsT=ident[:], rhs=f3[:, j, :],
                            start=(j == 0), stop=(j == K - 1))
                    # flush: psum (p, (a z w)) -> sbuf (p, (w a z))
                    o_t = aoutp.tile([128, 512], BF16D, tag="o")
                    accv = acc[:].rearrange("p (a z w) -> p w a z", a=4, z=4)
                    ov = o_t[:].rearrange("p (w a z) -> p w a z", w=32, a=4)
                    nc.scalar.activation(
                        out=ov[:], in_=accv[:], func=mybir.ActivationFunctionType.Copy)
                    nc.sync.dma_start(
                        out=ang_out[g * 65536:(g + 1) * 65536].rearrange("(p f) -> p f", p=128),
                        in_=o_t[:])
    nc.compile()
    return nc


# --------------------------------------------------------------------------
# entry point
# --------------------------------------------------------------------------

def _conv_table():
    conv = np.zeros(100, dtype=np.int32)
    for i, z in enumerate([1, 6, 7, 8]):
        conv[z] = i
    return conv


def _triu_table():
    s1, s2 = np.triu_indices(NUM_SPECIES, 0)
    triu = np.zeros((NUM_SPECIES, NUM_SPECIES), dtype=np.int32)
    triu[s1, s2] = np.arange(s1.shape[0], dtype=np.int32)
    triu[s2, s1] = triu[s1, s2]
    return triu


def kernel(ecfp, distances, switch, angles, ang_distances, ang_switch,
           species, bond_order, edge_src, edge_dst, ang_edge_dst,
           central_atom, angle_src, angle_dst):
    ecfp = np.asarray(ecfp, dtype=np.float32)
    distances = np.asarray(distances, dtype=np.float32)
    switch = np.asarray(switch, dtype=np.float32)
    angles = np.asarray(angles, dtype=np.float32)
    ang_distances = np.asarray(ang_distances, dtype=np.float32)
    ang_switch = np.asarray(ang_switch, dtype=np.float32)
    species = np.asarray(species, dtype=np.int32)
    bond_order = np.asarray(bond_order, dtype=np.int32)
    edge_src = np.asarray(edge_src, dtype=np.int32)
    edge_dst = np.asarray(edge_dst, dtype=np.int32)
    ang_edge_dst = np.asarray(ang_edge_dst, dtype=np.int32)
    central_atom = np.asarray(central_atom, dtype=np.int32)
    angle_src = np.asarray(angle_src, dtype=np.int32)
    angle_dst = np.asarray(angle_dst, dtype=np.int32)

    conv = _conv_table()
    triu = _triu_table()
    spec = conv[species]

    weights_bo = np.array([1.0, 1.5, 2.0, 0.5, 3.0, 0.25], dtype=np.float32)
    bbit = (weights_bo[bond_order] < 1.0).astype(np.int32)
    core_e = edge_src // ATOMS_PER_CORE
    rad_row = ((edge_src % ATOMS_PER_CORE) * NUM_SPECIES + spec[edge_dst]) * 2 + bbit

    idest = spec[ang_edge_dst]
    pairspec = triu[idest[angle_src], idest[angle_dst]]
    core_p = central_atom // ATOMS_PER_CORE
    ang_row = (central_atom % ATOMS_PER_CORE) * NUM_PAIR + pairspec

    rad_plans, ang_plans = [], []
    for c in range(N_CORES):
        rad_plans.append(_plan(rad_row[core_e == c], RAD_ROWS))
        ang_plans.append(_plan(ang_row[core_p == c], ANG_ROWS))
    ngr = max(len(p["ks"]) for p in rad_plans)
    nga = max(len(p["ks"]) for p in ang_plans)
    rad_ks = [max((p["ks"][g] if g < len(p["ks"]) else 0) for p in rad_plans)
              for g in range(ngr)]
    ang_ks = [max((p["ks"][g] if g < len(p["ks"]) else 0) for p in ang_plans)
              for g in range(nga)]

    in_maps = []
    for c in range(N_CORES):
        me = core_e == c
        slot, total_r = _slots(rad_plans[c], rad_ks)
        d_sl = np.ones(total_r, dtype=np.float32)
        c_sl = np.zeros(total_r, dtype=BF16)
        d_sl[slot] = distances[me]
        c_sl[slot] = (0.25 * switch[me]).astype(BF16)

        mp = core_p == c
        slot_a, total_a = _slots(ang_plans[c], ang_ks)
        th_sl = np.ones(total_a, dtype=np.float32)
        ds_sl = np.ones(total_a, dtype=np.float32)
        dt_sl = np.ones(total_a, dtype=np.float32)
        w_sl = np.full(total_a, 1e-35, dtype=np.float32)
        asrc = angle_src[mp]
        adst = angle_dst[mp]
        th_sl[slot_a] = angles[mp]
        ds_sl[slot_a] = ang_distances[asrc]
        dt_sl[slot_a] = ang_distances[adst]
        w_sl[slot_a] = np.maximum(ang_switch[asrc] * ang_switch[adst], 1e-35)
        in_maps.append(dict(
            rad_d=d_sl, rad_c=c_sl, ang_th=th_sl, ang_ds=ds_sl, ang_dt=dt_sl,
            ang_w=w_sl.astype(BF16)))

    nc = build_kernel(rad_ks, ang_ks)
    trace = bool(int(os.environ.get("KERNEL_TRACE", "0")))
    if trace:
        try:
            import antenv.axon_hooks  # noqa: F401
        except ImportError:
            try:
                import sys
                import types
                from trn_agent_boot.trn_boot import _ntff_profile_via_ctypes
                mod = types.ModuleType("antenv.axon_hooks")
                mod._hook = _ntff_profile_via_ctypes("/opt/axon/libaxon_pjrt.so")
                mod.get_axon_ntff_profile_hook = lambda: mod._hook
                mod.set_axon_ntff_profile_hook = lambda h: setattr(mod, "_hook", h)
                sys.modules["antenv.axon_hooks"] = mod
            except Exception as e:
                print(f"ntff hook shim failed ({e}); running untraced")
                trace = False
    res = run_bass_kernel_spmd(nc, in_maps, core_ids=list(range(N_CORES)),
                               trace=trace)
    if trace and res.exec_time_ns is not None:
        kernel.last_exec_time_ns = res.exec_time_ns
        print(f"HW exec time: {res.exec_time_ns} ns")

    out = np.zeros((N_ATOMS, ECFP_DIM + 128 + 160), dtype=np.float32)
    out[:, :ECFP_DIM] = ecfp
    for c in range(N_CORES):
        a0 = c * ATOMS_PER_CORE
        tab_r = _unshard_table(res.results[c]["rad_out"], rad_plans[c], rad_ks, RAD_ROWS)
        tr = tab_r.reshape(ATOMS_PER_CORE, NUM_SPECIES, 2, 16)
        out[a0:a0 + ATOMS_PER_CORE, 16:144] = \
            tr.transpose(0, 1, 3, 2).reshape(ATOMS_PER_CORE, 128)
        tab_a = _unshard_table(res.results[c]["ang_out"], ang_plans[c], ang_ks, ANG_ROWS)
        out[a0:a0 + ATOMS_PER_CORE, 144:304] = tab_a.reshape(ATOMS_PER_CORE, 160)
    return out
